# revision 25
# baseline (speedup 1.0000x reference)
"""SAGEConv-style GNN message passing on 8 Trainium2 NeuronCores.

out = (mean_{j in N(i)} x_j) @ W_l + b_l + x_i @ W_r
with N(i) defined by edge_index ([2, E]: src=row0, dst=row1), mean over
in-edges (segment mean by dst), N=100000 nodes, E=6400000 edges.

Distribution: shard by DESTINATION node range — core c owns nodes
[c*12500, (c+1)*12500) and receives exactly the edges targeting its range.
No collective needed.

Device algorithm (v2 — on-chip gather, no per-edge DMA):
  The src space is split into NS=20 sub-tables of 5000 nodes.  Each core's
  edges are bucketed host-side by (sub-table s, dst-group g) where the
  core's 12500 dst nodes are split into 8 groups of 1664; within a bucket
  edges stay dst-sorted.  Per stream s:
    1. DMA the feature-major sub-table [128, 5008] (partition 16g+r holds
       feature r of the sub-table, replicated per group) and int16 edge
       src indices.
    2. gpsimd.ap_gather pulls every edge's src features in ONE instruction
       per stream (each 16-partition group gathers its own edge slice).
    3. Unsegmented fp32 cumsum along the edge axis (tensor_tensor_scan,
       data0 = ones broadcast via a step-0 AP).
    4. gpsimd.ap_gather of the cumsum at per-node segment-close positions;
       consecutive differences give per-node per-stream partial sums.
       Gathered tiles are accumulated across streams (sum of diffs =
       diff of sums) into GACC.
  Epilogue: SUM = diff(GACC); degree from shipped close/prev close index
  tensors (device subtract); out = (SUM @ W_l) * rcp + x @ W_r + b_l via
  PE matmuls (features already partition-major) + one STT per 128-node
  tile.  Host work is layout only: bucketing/sorting edges, packing int16
  index tensors, replicating weights.
"""

import numpy as np

import concourse.bass as bass
import concourse.tile as tile
from concourse import bacc, mybir

# ---------------------------------------------------------------- config
N_NODES = 100000
N_EDGES = 6400000
IN_DIM = 10
HIDDEN = 16
N_CORES = 8

NODES_PC = N_NODES // N_CORES   # 12500 dst nodes per core
GROUPS = 8                      # dst groups per core (16 partitions each)
NPG = 1664                      # dst nodes per group (13*128)
NPAD = GROUPS * NPG             # 13312 padded dst nodes per core
TILES = NPG // 128              # 13 epilogue tiles per group
NJ = NPAD // 128                # 104 output cols per partition (N2 layout)

NS = 16                         # src sub-tables (streams)
SUB = N_NODES // NS             # 6250 src nodes per sub-table
TABW = SUB + 8                  # table width: 6250 rows + zero row + pad
ZROW = SUB                      # zero-row index (gathers 0.0)
LMAX = 7168                     # max edge slots per (stream, group)
SIW = LMAX // 16                # src idx tile free width (DRAM layout)
NCL = 1680                      # close entries per group: 105*16
CIW = NCL // 16


# ---------------------------------------------------------------- device
def build_program(num_devices=N_CORES, Ls=None):
    if Ls is None:
        Ls = _CACHE.get("Ls", [LMAX] * NS)
    P = 128
    nc = bacc.Bacc("TRN2", target_bir_lowering=False, debug=False,
                   num_devices=num_devices)

    xtab = nc.dram_tensor("xtab", [NS, P, TABW], mybir.dt.float32,
                          kind="ExternalInput")
    srcidx = nc.dram_tensor("srcidx", [NS, P, SIW], mybir.dt.int16,
                            kind="ExternalInput")
    cidx = nc.dram_tensor("cidx", [NS, P, CIW], mybir.dt.int16,
                          kind="ExternalInput")
    closen2 = nc.dram_tensor("closen2", [NS, P, NJ], mybir.dt.float32,
                             kind="ExternalInput")
    prevn2 = nc.dram_tensor("prevn2", [NS, P, NJ], mybir.dt.float32,
                            kind="ExternalInput")
    xsht = nc.dram_tensor("xsht", [P, NPG], mybir.dt.float32,
                          kind="ExternalInput")
    # weights stacked per 32-partition quadrant: cols 0:16 = W_l for the even
    # group of the pair, 16:32 = W_l for the odd group, 32:48/48:64 likewise
    # for [W_r; b_l] (row 10 = b_l meets the ones row of xsht).
    w4 = nc.dram_tensor("w4", [P, 4 * HIDDEN], mybir.dt.float32,
                        kind="ExternalInput")
    out_d = nc.dram_tensor("out", [P, NJ * HIDDEN], mybir.dt.float32,
                           kind="ExternalOutput")

    with tile.TileContext(nc) as tc:
        with (
            tc.tile_pool(name="tab", bufs=2) as tab_pool,
            tc.tile_pool(name="msg", bufs=2) as msg_pool,
            tc.tile_pool(name="scan", bufs=1) as scan_pool,
            tc.tile_pool(name="idx", bufs=2) as idx_pool,
            tc.tile_pool(name="g", bufs=2) as g_pool,
            tc.tile_pool(name="epi", bufs=1) as epi_pool,
            tc.tile_pool(name="psum", bufs=8,
                         space=bass.MemorySpace.PSUM) as psum_pool,
        ):
            # prefetch stream 0's table + indices ahead of the statics so the
            # first gather starts as early as possible
            tab0_t = tab_pool.tile([P, TABW], mybir.dt.float32, tag="tab")
            nc.sync.dma_start(tab0_t[:], xtab.ap()[0])
            si0_t = idx_pool.tile([P, SIW], mybir.dt.int16, tag="si")
            nc.sync.dma_start(si0_t[:, :Ls[0] // 16],
                              srcidx.ap()[0][:, :Ls[0] // 16])

            # ---- static tiles
            ones_t = epi_pool.tile([P, 1], mybir.dt.float32)
            nc.vector.memset(ones_t[:], 1.0)
            gacc_t = epi_pool.tile([P, NCL], mybir.dt.float32)
            nc.vector.memset(gacc_t[:], 0.0)
            dacc_t = epi_pool.tile([P, NJ], mybir.dt.float32)
            nc.vector.memset(dacc_t[:], 0.0)
            pacc_t = epi_pool.tile([P, NJ], mybir.dt.float32)
            nc.vector.memset(pacc_t[:], 0.0)
            xsht_t = epi_pool.tile([P, NPG], mybir.dt.float32)
            nc.sync.dma_start(xsht_t[:], xsht.ap())
            w4_t = epi_pool.tile([P, 4 * HIDDEN], mybir.dt.float32)
            nc.sync.dma_start(w4_t[:], w4.ap())

            # x @ W_r (+ b_l via the ones row) matmuls run during the streams
            # (PE is otherwise idle); results stay resident in PSUM: one bank
            # per group, cols [0,208) reserved for the tail's W_l terms.
            gw = TILES * HIDDEN
            pg_tiles = []
            for g in range(GROUPS):
                q, lo = divmod(g, 2)
                wr_cols = slice(HIDDEN * (2 + lo), HIDDEN * (3 + lo))
                rows = slice(32 * q, 32 * (q + 1))
                pg = psum_pool.tile([P, 2 * gw], mybir.dt.float32, tag="pg")
                pg_tiles.append(pg)
                for t in range(TILES):
                    nc.tensor.matmul(
                        pg[:, gw + HIDDEN * t:gw + HIDDEN * (t + 1)],
                        xsht_t[rows, 128 * t:128 * (t + 1)],
                        w4_t[rows, wr_cols],
                        tile_position=(32 * q, 0))

            # ---- streams
            for s in range(NS):
                ls = Ls[s]
                if s == 0:
                    tab_t, si_t = tab0_t, si0_t
                else:
                    tab_t = tab_pool.tile([P, TABW], mybir.dt.float32,
                                          tag="tab")
                    nc.sync.dma_start(tab_t[:], xtab.ap()[s])
                    si_t = idx_pool.tile([P, SIW], mybir.dt.int16, tag="si")
                    nc.sync.dma_start(si_t[:, :ls // 16],
                                      srcidx.ap()[s][:, :ls // 16])
                ci_t = idx_pool.tile([P, CIW], mybir.dt.int16, tag="ci")
                nc.sync.dma_start(ci_t[:], cidx.ap()[s])
                cn_t = idx_pool.tile([P, NJ], mybir.dt.float32, tag="cn")
                nc.sync.dma_start(cn_t[:], closen2.ap()[s])
                pn_t = idx_pool.tile([P, NJ], mybir.dt.float32, tag="pn")
                nc.sync.dma_start(pn_t[:], prevn2.ap()[s])

                msg_t = msg_pool.tile([P, LMAX], mybir.dt.float32, tag="msg")
                nc.gpsimd.ap_gather(
                    out_ap=msg_t[:, :ls].rearrange("p (n o) -> p n o", o=1),
                    in_ap=tab_t[:].rearrange("p (n o) -> p n o", o=1),
                    idxs_ap=si_t[:, :ls // 16],
                    channels=P, num_elems=TABW, d=1, num_idxs=ls)

                scan_t = scan_pool.tile([P, LMAX], mybir.dt.float32,
                                        tag="scan")
                d0 = bass.AP(ones_t[:].tensor, ones_t[:].offset,
                             [ones_t[:].ap[0], [0, ls]])
                nc.vector.tensor_tensor_scan(
                    out=scan_t[:, :ls], data0=d0, data1=msg_t[:, :ls],
                    initial=0.0,
                    op0=mybir.AluOpType.mult, op1=mybir.AluOpType.add)

                g_t = g_pool.tile([P, NCL], mybir.dt.float32, tag="g")
                nc.gpsimd.ap_gather(
                    out_ap=g_t[:].rearrange("p (n o) -> p n o", o=1),
                    in_ap=scan_t[:, :ls].rearrange("p (n o) -> p n o", o=1),
                    idxs_ap=ci_t[:],
                    channels=P, num_elems=ls, d=1, num_idxs=NCL)
                nc.vector.tensor_add(out=gacc_t[:], in0=gacc_t[:], in1=g_t[:])
                nc.vector.tensor_add(out=dacc_t[:], in0=dacc_t[:], in1=cn_t[:])
                nc.vector.tensor_add(out=pacc_t[:], in0=pacc_t[:], in1=pn_t[:])

            # ---- epilogue
            sum_t = epi_pool.tile([P, NPG], mybir.dt.float32)
            nc.vector.tensor_tensor(
                out=sum_t[:], in0=gacc_t[:, 1:NPG + 1], in1=gacc_t[:, 0:NPG],
                op=mybir.AluOpType.subtract)
            deg_t = epi_pool.tile([P, NJ], mybir.dt.float32)
            nc.vector.tensor_tensor(
                out=deg_t[:], in0=dacc_t[:], in1=pacc_t[:],
                op=mybir.AluOpType.subtract)
            nc.vector.tensor_scalar_max(deg_t[:], deg_t[:], 1.0)
            rcp_t = epi_pool.tile([P, NJ], mybir.dt.float32)
            nc.vector.reciprocal(rcp_t[:], deg_t[:])

            out_t = epi_pool.tile([P, NJ * HIDDEN], mybir.dt.float32)
            t1_t = epi_pool.tile([P, TILES * HIDDEN], mybir.dt.float32)
            rcpx_t = epi_pool.tile([P, TILES * HIDDEN], mybir.dt.float32)
            for g in range(GROUPS):
                q, lo = divmod(g, 2)
                wl_cols = slice(HIDDEN * lo, HIDDEN * (lo + 1))
                rows = slice(32 * q, 32 * (q + 1))
                pg = pg_tiles[g]
                for t in range(TILES):
                    nc.tensor.matmul(
                        pg[:, HIDDEN * t:HIDDEN * (t + 1)],
                        sum_t[rows, 128 * t:128 * (t + 1)],
                        w4_t[rows, wl_cols],
                        tile_position=(32 * q, 0))
                # rcpx[:, 16t+h] = rcp[:, 13g+t]  (free-dim step-0 broadcast)
                rg = rcp_t[:, TILES * g:TILES * (g + 1)]
                rbc = bass.AP(rg.tensor, rg.offset,
                              [rg.ap[0], rg.ap[1], [0, HIDDEN]])
                nc.scalar.activation(
                    out=rcpx_t[:].rearrange("p (t h) -> p t h", h=HIDDEN),
                    in_=rbc, func=mybir.ActivationFunctionType.Copy)
                # out = p1 * rcpx + p2   (each op reads one PSUM operand)
                nc.vector.tensor_tensor(
                    out=t1_t[:], in0=pg[:, :gw], in1=rcpx_t[:],
                    op=mybir.AluOpType.mult)
                nc.vector.tensor_tensor(
                    out=out_t[:, gw * g:gw * (g + 1)],
                    in0=pg[:, gw:2 * gw], in1=t1_t[:],
                    op=mybir.AluOpType.add)
            nc.sync.dma_start(out_d.ap(), out_t[:])

    nc.compile()
    return nc


# ---------------------------------------------------------------- host
def prepare_inputs(x, edge_index, W_l, b_l, W_r):
    x = np.asarray(x, np.float32)
    W_l = np.asarray(W_l, np.float32)
    b_l = np.asarray(b_l, np.float32)
    W_r = np.asarray(W_r, np.float32)
    src = np.asarray(edge_index[0], np.int64)
    dst = np.asarray(edge_index[1], np.int64)

    # replicated feature-major sub-tables: [NS, 128, TABW]
    # partition 16g+r holds feature r of sub-table s (r<10), zeros otherwise;
    # col ZROW is the zero row.
    xtab = np.zeros((NS, 16, TABW), np.float32)
    for s in range(NS):
        xtab[s, :IN_DIM, :SUB] = x[s * SUB:(s + 1) * SUB].T
    xtab = np.ascontiguousarray(
        np.broadcast_to(xtab[:, None, :, :], (NS, GROUPS, 16, TABW))
    ).reshape(NS, 128, TABW)

    # weights: rows r<10 = W[r]; xsht row 10 is ones so the W_r block gets
    # b_l at row 10.  Stacked [32, 64] quadrant pattern tiled 4x -> [128, 64].
    wlb = np.zeros((16, HIDDEN), np.float32)
    wlb[:IN_DIM] = W_l
    wrb = np.zeros((16, HIDDEN), np.float32)
    wrb[:IN_DIM] = W_r
    wrb[IN_DIM] = b_l
    w32 = np.zeros((32, 4 * HIDDEN), np.float32)
    w32[:16, 0:16] = wlb
    w32[16:, 16:32] = wlb
    w32[:16, 32:48] = wrb
    w32[16:, 48:64] = wrb
    w4 = np.tile(w32, (4, 1))

    order = np.argsort(dst, kind="stable")
    dst_s = dst[order]
    src_s = src[order]
    core_bounds = np.searchsorted(dst_s, np.arange(0, N_NODES + 1, NODES_PC))
    ls_max = np.zeros(NS, np.int64)      # per-stream tight L across cores

    def wrap16(a):
        # [..., K*16] entry-order -> [..., 16, K] partition-wrapped
        shp = a.shape[:-1]
        k = a.shape[-1] // 16
        return np.swapaxes(a.reshape(*shp, k, 16), -1, -2)

    in_maps = []
    for c in range(N_CORES):
        lo, hi = core_bounds[c], core_bounds[c + 1]
        d_l = dst_s[lo:hi] - c * NODES_PC          # [Ec] in [0, 12500)
        sc = src_s[lo:hi]
        s_id = sc // SUB                            # stream
        g_id = d_l // NPG                           # dst group
        src_loc = (sc - s_id * SUB).astype(np.int16)
        key = (s_id * GROUPS + g_id).astype(np.int64)

        korder = np.argsort(key, kind="stable")     # dst order kept in bucket
        key_s = key[korder]
        srcl_s = src_loc[korder]
        dl_s = d_l[korder]

        cnts = np.bincount(key_s, minlength=NS * GROUPS)
        assert cnts.max() <= LMAX - 8, f"bucket overflow: {cnts.max()}"
        np.maximum(ls_max, cnts.reshape(NS, GROUPS).max(axis=1), out=ls_max)
        starts = np.zeros(NS * GROUPS, np.int64)
        np.cumsum(cnts[:-1], out=starts[1:])
        rank = np.arange(key_s.shape[0]) - starts[key_s]

        # src idx slots: col 0 zero slot, edges at cols 1..cnt, pads ZROW
        A = np.full((NS * GROUPS, LMAX), ZROW, np.int16)
        A[key_s, rank + 1] = srcl_s
        srcidx = wrap16(A.reshape(NS, GROUPS, LMAX)).reshape(NS, 128, SIW)

        # per-node close cols: cumcount within bucket (0 => zero slot)
        node_key = key_s * NPG + (dl_s - g_id[korder] * NPG)
        ncnt = np.bincount(node_key, minlength=NS * GROUPS * NPG)
        closes = np.cumsum(
            ncnt.reshape(NS * GROUPS, NPG), axis=1)     # [160, 1664]
        entries = np.full((NS * GROUPS, NCL), 0, np.int16)
        entries[:, 1:NPG + 1] = closes
        entries[:, NPG + 1:] = closes[:, -1:]
        cidx = wrap16(entries.reshape(NS, GROUPS, NCL)).reshape(NS, 128, CIW)

        # N2 close/prev tensors (f32): node n at partition n%128, col n//128
        cl3 = closes.reshape(NS, GROUPS, NPG).astype(np.float32)
        pr3 = np.concatenate(
            [np.zeros((NS, GROUPS, 1), np.float32), cl3[:, :, :-1]], axis=2)
        # [NS, GROUPS*NPG] -> [NS, NJ, 128] -> [NS, 128, NJ]
        closen2 = np.swapaxes(cl3.reshape(NS, NPAD // 128, 128), 1, 2).copy()
        prevn2 = np.swapaxes(pr3.reshape(NS, NPAD // 128, 128), 1, 2).copy()

        # xsht: partition 16g+r = feature r (row 10 = ones) of nodes
        # g*1664..; pad nodes zero (row 10 stays 1 -> out = b_l there, unused)
        xsh = np.zeros((GROUPS, 16, NPG), np.float32)
        base = c * NODES_PC
        for g in range(GROUPS):
            n0 = g * NPG
            n1 = min((g + 1) * NPG, NODES_PC)
            if n1 > n0:
                xsh[g, :IN_DIM, :n1 - n0] = x[base + n0:base + n1].T
        xsh[:, IN_DIM, :] = 1.0
        xsht = xsh.reshape(128, NPG)

        in_maps.append({
            "xtab": xtab, "srcidx": srcidx, "cidx": cidx,
            "closen2": closen2, "prevn2": prevn2, "xsht": xsht,
            "w4": w4,
        })
    _CACHE.setdefault(
        "Ls", [int(-(-(v + 2) // 16) * 16) for v in ls_max])
    return in_maps


# ---------------------------------------------------------------- runner
class SpmdRunner:
    def __init__(self, nc, n_cores):
        import jax
        from jax.sharding import Mesh, PartitionSpec
        from jax.experimental.shard_map import shard_map
        from concourse.bass2jax import (
            _bass_exec_p, install_neuronx_cc_hook, partition_id_tensor)

        install_neuronx_cc_hook()
        self.n_cores = n_cores
        pname = nc.partition_id_tensor.name if nc.partition_id_tensor else None
        in_names, out_names, out_avals, zero_outs = [], [], [], []
        for alloc in nc.m.functions[0].allocations:
            if not isinstance(alloc, mybir.MemoryLocationSet):
                continue
            name = alloc.memorylocations[0].name
            if alloc.kind == "ExternalInput":
                if name != pname:
                    in_names.append(name)
            elif alloc.kind == "ExternalOutput":
                out_names.append(name)
                shape = tuple(alloc.tensor_shape)
                dt_np = mybir.dt.np(alloc.dtype)
                out_avals.append(jax.core.ShapedArray(shape, dt_np))
                zero_outs.append(np.zeros(shape, dt_np))
        self.in_names, self.out_names = in_names, out_names
        self.zero_outs = zero_outs
        n_params, n_outs = len(in_names), len(out_names)
        all_names = in_names + out_names + ([pname] if pname else [])

        def _body(*args):
            operands = list(args)
            if pname is not None:
                operands.append(partition_id_tensor())
            return tuple(_bass_exec_p.bind(
                *operands, out_avals=tuple(out_avals),
                in_names=tuple(all_names), out_names=tuple(out_names),
                lowering_input_output_aliases=(),
                sim_require_finite=True, sim_require_nnan=True, nc=nc))

        devices = jax.devices()[:n_cores]
        mesh = Mesh(np.asarray(devices), ("core",))
        self._mesh = mesh
        specs_in = (PartitionSpec("core"),) * (n_params + n_outs)
        specs_out = (PartitionSpec("core"),) * n_outs
        self._fn = jax.jit(
            shard_map(_body, mesh=mesh, in_specs=specs_in,
                      out_specs=specs_out, check_rep=False),
            keep_unused=True)
        self._jax = jax

    def prepare(self, in_maps):
        per = [[np.asarray(m[n]) for n in self.in_names] for m in in_maps]
        cat = [np.concatenate([per[c][i] for c in range(self.n_cores)], axis=0)
               for i in range(len(self.in_names))]
        cat += [np.concatenate([z] * self.n_cores, axis=0)
                for z in self.zero_outs]
        return cat

    def device_put(self, args):
        """Ship prepared args to the devices once (for repeat timing)."""
        import jax
        from jax.sharding import NamedSharding, PartitionSpec
        mesh = self._mesh
        sh = NamedSharding(mesh, PartitionSpec("core"))
        out = [jax.device_put(a, sh) for a in args]
        jax.block_until_ready(out)
        return out

    def run(self, args):
        outs = self._fn(*args)
        self._jax.block_until_ready(outs)
        return outs

    def results(self, outs):
        res = [dict() for _ in range(self.n_cores)]
        for i, name in enumerate(self.out_names):
            for c, part in enumerate(
                    np.split(np.asarray(outs[i]), self.n_cores, axis=0)):
                res[c][name] = part
        return res


_CACHE = {}


def kernel(x, edge_index, W_l, b_l, W_r):
    in_maps = prepare_inputs(x, edge_index, W_l, b_l, W_r)
    if "runner" not in _CACHE:
        nc = build_program()
        _CACHE["runner"] = SpmdRunner(nc, N_CORES)
    runner = _CACHE["runner"]
    args = runner.prepare(in_maps)
    res = runner.results(runner.run(args))
    out = np.empty((N_NODES, HIDDEN), np.float32)
    for c in range(N_CORES):
        od = res[c]["out"]                        # [128, NJ*16]
        # node n at partition n%128, col-block n//128
        full = od.reshape(128, NJ, HIDDEN).transpose(1, 0, 2).reshape(
            NPAD, HIDDEN)
        out[c * NODES_PC:(c + 1) * NODES_PC] = full[:NODES_PC]
    return out


# revision 30
# speedup vs baseline: 1.0544x; 1.0544x over previous
"""SAGEConv-style GNN message passing on 8 Trainium2 NeuronCores.

out = (mean_{j in N(i)} x_j) @ W_l + b_l + x_i @ W_r
with N(i) defined by edge_index ([2, E]: src=row0, dst=row1), mean over
in-edges (segment mean by dst), N=100000 nodes, E=6400000 edges.

Distribution: shard by DESTINATION node range — core c owns nodes
[c*12500, (c+1)*12500) and receives exactly the edges targeting its range.
No collective needed.

Device algorithm (v2 — on-chip gather, no per-edge DMA):
  The src space is split into NS=20 sub-tables of 5000 nodes.  Each core's
  edges are bucketed host-side by (sub-table s, dst-group g) where the
  core's 12500 dst nodes are split into 8 groups of 1664; within a bucket
  edges stay dst-sorted.  Per stream s:
    1. DMA the feature-major sub-table [128, 5008] (partition 16g+r holds
       feature r of the sub-table, replicated per group) and int16 edge
       src indices.
    2. gpsimd.ap_gather pulls every edge's src features in ONE instruction
       per stream (each 16-partition group gathers its own edge slice).
    3. Unsegmented fp32 cumsum along the edge axis (tensor_tensor_scan,
       data0 = ones broadcast via a step-0 AP).
    4. gpsimd.ap_gather of the cumsum at per-node segment-close positions;
       consecutive differences give per-node per-stream partial sums.
       Gathered tiles are accumulated across streams (sum of diffs =
       diff of sums) into GACC.
  Epilogue: SUM = diff(GACC); degree from shipped close/prev close index
  tensors (device subtract); out = (SUM @ W_l) * rcp + x @ W_r + b_l via
  PE matmuls (features already partition-major) + one STT per 128-node
  tile.  Host work is layout only: bucketing/sorting edges, packing int16
  index tensors, replicating weights.
"""

import numpy as np

import concourse.bass as bass
import concourse.tile as tile
from concourse import bacc, mybir

# ---------------------------------------------------------------- config
N_NODES = 100000
N_EDGES = 6400000
IN_DIM = 10
HIDDEN = 16
N_CORES = 8

NODES_PC = N_NODES // N_CORES   # 12500 dst nodes per core
GROUPS = 8                      # dst groups per core (16 partitions each)
NPG = 1664                      # dst node SLOTS per group (13*128)
GN = 1568                       # real dst nodes per group (balanced load)
NPAD = GROUPS * NPG             # 13312 padded dst nodes per core
TILES = NPG // 128              # 13 epilogue tiles per group
NJ = NPAD // 128                # 104 output cols per partition (N2 layout)

NS = 16                         # src sub-tables (streams)
SUB = N_NODES // NS             # 6250 src nodes per sub-table
TABW = SUB + 8                  # table width: 6250 rows + zero row + pad
ZROW = SUB                      # zero-row index (gathers 0.0)
LMAX = 7168                     # max edge slots per (stream, group)
SIW = LMAX // 16                # src idx tile free width (DRAM layout)
NCL = 1680                      # close entries per group: 105*16
CIW = NCL // 16


# ---------------------------------------------------------------- device
def build_program(num_devices=N_CORES, Ls=None):
    if Ls is None:
        Ls = _CACHE.get("Ls", [LMAX] * NS)
    P = 128
    nc = bacc.Bacc("TRN2", target_bir_lowering=False, debug=False,
                   num_devices=num_devices)

    xtab = nc.dram_tensor("xtab", [NS, P, TABW], mybir.dt.float32,
                          kind="ExternalInput")
    srcidx = nc.dram_tensor("srcidx", [NS, P, SIW], mybir.dt.int16,
                            kind="ExternalInput")
    cidx = nc.dram_tensor("cidx", [NS, P, CIW], mybir.dt.int16,
                          kind="ExternalInput")
    closen2 = nc.dram_tensor("closen2", [NS, P, NJ], mybir.dt.float32,
                             kind="ExternalInput")
    prevn2 = nc.dram_tensor("prevn2", [NS, P, NJ], mybir.dt.float32,
                            kind="ExternalInput")
    xsht = nc.dram_tensor("xsht", [P, NPG], mybir.dt.float32,
                          kind="ExternalInput")
    # weights stacked per 32-partition quadrant: cols 0:16 = W_l for the even
    # group of the pair, 16:32 = W_l for the odd group, 32:48/48:64 likewise
    # for [W_r; b_l] (row 10 = b_l meets the ones row of xsht).
    w4 = nc.dram_tensor("w4", [P, 4 * HIDDEN], mybir.dt.float32,
                        kind="ExternalInput")
    out_d = nc.dram_tensor("out", [P, NJ * HIDDEN], mybir.dt.float32,
                           kind="ExternalOutput")

    with tile.TileContext(nc) as tc:
        with (
            tc.tile_pool(name="tab", bufs=2) as tab_pool,
            tc.tile_pool(name="msg", bufs=2) as msg_pool,
            tc.tile_pool(name="scan", bufs=1) as scan_pool,
            tc.tile_pool(name="idx", bufs=2) as idx_pool,
            tc.tile_pool(name="g", bufs=2) as g_pool,
            tc.tile_pool(name="epi", bufs=1) as epi_pool,
            tc.tile_pool(name="psum", bufs=8,
                         space=bass.MemorySpace.PSUM) as psum_pool,
        ):
            # prefetch stream 0's table + indices ahead of the statics so the
            # first gather starts as early as possible
            tab0_t = tab_pool.tile([P, TABW], mybir.dt.float32, tag="tab")
            nc.sync.dma_start(tab0_t[:], xtab.ap()[0])
            si0_t = idx_pool.tile([P, SIW], mybir.dt.int16, tag="si")
            nc.sync.dma_start(si0_t[:, :Ls[0] // 16],
                              srcidx.ap()[0][:, :Ls[0] // 16])

            # ---- static tiles
            ones_t = epi_pool.tile([P, 1], mybir.dt.float32)
            nc.vector.memset(ones_t[:], 1.0)
            gacc_t = epi_pool.tile([P, NCL], mybir.dt.float32)
            nc.vector.memset(gacc_t[:], 0.0)
            dacc_t = epi_pool.tile([P, NJ], mybir.dt.float32)
            nc.vector.memset(dacc_t[:], 0.0)
            pacc_t = epi_pool.tile([P, NJ], mybir.dt.float32)
            nc.vector.memset(pacc_t[:], 0.0)
            xsht_t = epi_pool.tile([P, NPG], mybir.dt.float32)
            nc.sync.dma_start(xsht_t[:], xsht.ap())
            w4_t = epi_pool.tile([P, 4 * HIDDEN], mybir.dt.float32)
            nc.sync.dma_start(w4_t[:], w4.ap())

            # x @ W_r (+ b_l via the ones row) matmuls run during the streams
            # (PE is otherwise idle); results stay resident in PSUM: one bank
            # per group, cols [0,208) reserved for the tail's W_l terms.
            gw = TILES * HIDDEN
            pg_tiles = []
            for g in range(GROUPS):
                q, lo = divmod(g, 2)
                wr_cols = slice(HIDDEN * (2 + lo), HIDDEN * (3 + lo))
                rows = slice(32 * q, 32 * (q + 1))
                pg = psum_pool.tile([P, 2 * gw], mybir.dt.float32, tag="pg")
                pg_tiles.append(pg)
                for t in range(TILES):
                    nc.tensor.matmul(
                        pg[:, gw + HIDDEN * t:gw + HIDDEN * (t + 1)],
                        xsht_t[rows, 128 * t:128 * (t + 1)],
                        w4_t[rows, wr_cols],
                        tile_position=(32 * q, 0))

            # ---- streams
            for s in range(NS):
                ls = Ls[s]
                if s == 0:
                    tab_t, si_t = tab0_t, si0_t
                else:
                    tab_t = tab_pool.tile([P, TABW], mybir.dt.float32,
                                          tag="tab")
                    nc.sync.dma_start(tab_t[:], xtab.ap()[s])
                    si_t = idx_pool.tile([P, SIW], mybir.dt.int16, tag="si")
                    nc.sync.dma_start(si_t[:, :ls // 16],
                                      srcidx.ap()[s][:, :ls // 16])
                ci_t = idx_pool.tile([P, CIW], mybir.dt.int16, tag="ci")
                nc.sync.dma_start(ci_t[:], cidx.ap()[s])
                cn_t = idx_pool.tile([P, NJ], mybir.dt.float32, tag="cn")
                nc.sync.dma_start(cn_t[:], closen2.ap()[s])
                pn_t = idx_pool.tile([P, NJ], mybir.dt.float32, tag="pn")
                nc.sync.dma_start(pn_t[:], prevn2.ap()[s])

                msg_t = msg_pool.tile([P, LMAX], mybir.dt.float32, tag="msg")
                nc.gpsimd.ap_gather(
                    out_ap=msg_t[:, :ls].rearrange("p (n o) -> p n o", o=1),
                    in_ap=tab_t[:].rearrange("p (n o) -> p n o", o=1),
                    idxs_ap=si_t[:, :ls // 16],
                    channels=P, num_elems=TABW, d=1, num_idxs=ls)

                scan_t = scan_pool.tile([P, LMAX], mybir.dt.float32,
                                        tag="scan")
                d0 = bass.AP(ones_t[:].tensor, ones_t[:].offset,
                             [ones_t[:].ap[0], [0, ls]])
                nc.vector.tensor_tensor_scan(
                    out=scan_t[:, :ls], data0=d0, data1=msg_t[:, :ls],
                    initial=0.0,
                    op0=mybir.AluOpType.mult, op1=mybir.AluOpType.add)

                g_t = g_pool.tile([P, NCL], mybir.dt.float32, tag="g")
                nc.gpsimd.ap_gather(
                    out_ap=g_t[:].rearrange("p (n o) -> p n o", o=1),
                    in_ap=scan_t[:, :ls].rearrange("p (n o) -> p n o", o=1),
                    idxs_ap=ci_t[:],
                    channels=P, num_elems=ls, d=1, num_idxs=NCL)
                nc.vector.tensor_add(out=gacc_t[:], in0=gacc_t[:], in1=g_t[:])
                nc.vector.tensor_add(out=dacc_t[:], in0=dacc_t[:], in1=cn_t[:])
                nc.vector.tensor_add(out=pacc_t[:], in0=pacc_t[:], in1=pn_t[:])

            # ---- epilogue
            sum_t = epi_pool.tile([P, NPG], mybir.dt.float32)
            nc.vector.tensor_tensor(
                out=sum_t[:], in0=gacc_t[:, 1:NPG + 1], in1=gacc_t[:, 0:NPG],
                op=mybir.AluOpType.subtract)
            deg_t = epi_pool.tile([P, NJ], mybir.dt.float32)
            nc.vector.tensor_tensor(
                out=deg_t[:], in0=dacc_t[:], in1=pacc_t[:],
                op=mybir.AluOpType.subtract)
            nc.vector.tensor_scalar_max(deg_t[:], deg_t[:], 1.0)
            rcp_t = epi_pool.tile([P, NJ], mybir.dt.float32)
            nc.vector.reciprocal(rcp_t[:], deg_t[:])

            out_t = epi_pool.tile([P, NJ * HIDDEN], mybir.dt.float32)
            t1_t = epi_pool.tile([P, TILES * HIDDEN], mybir.dt.float32)
            rcpx_t = epi_pool.tile([P, TILES * HIDDEN], mybir.dt.float32)
            for g in range(GROUPS):
                q, lo = divmod(g, 2)
                wl_cols = slice(HIDDEN * lo, HIDDEN * (lo + 1))
                rows = slice(32 * q, 32 * (q + 1))
                pg = pg_tiles[g]
                for t in range(TILES):
                    nc.tensor.matmul(
                        pg[:, HIDDEN * t:HIDDEN * (t + 1)],
                        sum_t[rows, 128 * t:128 * (t + 1)],
                        w4_t[rows, wl_cols],
                        tile_position=(32 * q, 0))
                # rcpx[:, 16t+h] = rcp[:, 13g+t]  (free-dim step-0 broadcast)
                rg = rcp_t[:, TILES * g:TILES * (g + 1)]
                rbc = bass.AP(rg.tensor, rg.offset,
                              [rg.ap[0], rg.ap[1], [0, HIDDEN]])
                nc.scalar.activation(
                    out=rcpx_t[:].rearrange("p (t h) -> p t h", h=HIDDEN),
                    in_=rbc, func=mybir.ActivationFunctionType.Copy)
                # out = p1 * rcpx + p2   (each op reads one PSUM operand)
                nc.vector.tensor_tensor(
                    out=t1_t[:], in0=pg[:, :gw], in1=rcpx_t[:],
                    op=mybir.AluOpType.mult)
                nc.vector.tensor_tensor(
                    out=out_t[:, gw * g:gw * (g + 1)],
                    in0=pg[:, gw:2 * gw], in1=t1_t[:],
                    op=mybir.AluOpType.add)
            nc.sync.dma_start(out_d.ap(), out_t[:])

    nc.compile()
    return nc


# ---------------------------------------------------------------- host
def prepare_inputs(x, edge_index, W_l, b_l, W_r):
    x = np.asarray(x, np.float32)
    W_l = np.asarray(W_l, np.float32)
    b_l = np.asarray(b_l, np.float32)
    W_r = np.asarray(W_r, np.float32)
    src = np.asarray(edge_index[0], np.int64)
    dst = np.asarray(edge_index[1], np.int64)

    # replicated feature-major sub-tables: [NS, 128, TABW]
    # partition 16g+r holds feature r of sub-table s (r<10), zeros otherwise;
    # col ZROW is the zero row.
    xtab = np.zeros((NS, 16, TABW), np.float32)
    for s in range(NS):
        xtab[s, :IN_DIM, :SUB] = x[s * SUB:(s + 1) * SUB].T
    xtab = np.ascontiguousarray(
        np.broadcast_to(xtab[:, None, :, :], (NS, GROUPS, 16, TABW))
    ).reshape(NS, 128, TABW)

    # weights: rows r<10 = W[r]; xsht row 10 is ones so the W_r block gets
    # b_l at row 10.  Stacked [32, 64] quadrant pattern tiled 4x -> [128, 64].
    wlb = np.zeros((16, HIDDEN), np.float32)
    wlb[:IN_DIM] = W_l
    wrb = np.zeros((16, HIDDEN), np.float32)
    wrb[:IN_DIM] = W_r
    wrb[IN_DIM] = b_l
    w32 = np.zeros((32, 4 * HIDDEN), np.float32)
    w32[:16, 0:16] = wlb
    w32[16:, 16:32] = wlb
    w32[:16, 32:48] = wrb
    w32[16:, 48:64] = wrb
    w4 = np.tile(w32, (4, 1))

    order = np.argsort(dst, kind="stable")
    dst_s = dst[order]
    src_s = src[order]
    core_bounds = np.searchsorted(dst_s, np.arange(0, N_NODES + 1, NODES_PC))
    ls_max = np.zeros(NS, np.int64)      # per-stream tight L across cores

    def wrap16(a):
        # [..., K*16] entry-order -> [..., 16, K] partition-wrapped
        shp = a.shape[:-1]
        k = a.shape[-1] // 16
        return np.swapaxes(a.reshape(*shp, k, 16), -1, -2)

    in_maps = []
    for c in range(N_CORES):
        lo, hi = core_bounds[c], core_bounds[c + 1]
        d_l = dst_s[lo:hi] - c * NODES_PC          # [Ec] in [0, 12500)
        sc = src_s[lo:hi]
        s_id = sc // SUB                            # stream
        g_id = d_l // GN                            # dst group (balanced)
        src_loc = (sc - s_id * SUB).astype(np.int16)
        key = (s_id * GROUPS + g_id).astype(np.int64)

        korder = np.argsort(key, kind="stable")     # dst order kept in bucket
        key_s = key[korder]
        srcl_s = src_loc[korder]
        dl_s = d_l[korder]

        cnts = np.bincount(key_s, minlength=NS * GROUPS)
        assert cnts.max() <= LMAX - 8, f"bucket overflow: {cnts.max()}"
        np.maximum(ls_max, cnts.reshape(NS, GROUPS).max(axis=1), out=ls_max)
        starts = np.zeros(NS * GROUPS, np.int64)
        np.cumsum(cnts[:-1], out=starts[1:])
        rank = np.arange(key_s.shape[0]) - starts[key_s]

        # src idx slots: col 0 zero slot, edges at cols 1..cnt, pads ZROW
        A = np.full((NS * GROUPS, LMAX), ZROW, np.int16)
        A[key_s, rank + 1] = srcl_s
        srcidx = wrap16(A.reshape(NS, GROUPS, LMAX)).reshape(NS, 128, SIW)

        # per-node close cols: cumcount within bucket (0 => zero slot)
        node_key = key_s * NPG + (dl_s - g_id[korder] * GN)
        ncnt = np.bincount(node_key, minlength=NS * GROUPS * NPG)
        closes = np.cumsum(
            ncnt.reshape(NS * GROUPS, NPG), axis=1)     # [160, 1664]
        entries = np.full((NS * GROUPS, NCL), 0, np.int16)
        entries[:, 1:NPG + 1] = closes
        entries[:, NPG + 1:] = closes[:, -1:]
        cidx = wrap16(entries.reshape(NS, GROUPS, NCL)).reshape(NS, 128, CIW)

        # N2 close/prev tensors (f32): node n at partition n%128, col n//128
        cl3 = closes.reshape(NS, GROUPS, NPG).astype(np.float32)
        pr3 = np.concatenate(
            [np.zeros((NS, GROUPS, 1), np.float32), cl3[:, :, :-1]], axis=2)
        # [NS, GROUPS*NPG] -> [NS, NJ, 128] -> [NS, 128, NJ]
        closen2 = np.swapaxes(cl3.reshape(NS, NPAD // 128, 128), 1, 2).copy()
        prevn2 = np.swapaxes(pr3.reshape(NS, NPAD // 128, 128), 1, 2).copy()

        # xsht: partition 16g+r = feature r (row 10 = ones) of nodes
        # g*GN..; pad slots zero (row 10 stays 1 -> out = b_l there, unused)
        xsh = np.zeros((GROUPS, 16, NPG), np.float32)
        base = c * NODES_PC
        for g in range(GROUPS):
            n0 = g * GN
            n1 = min((g + 1) * GN, NODES_PC)
            if n1 > n0:
                xsh[g, :IN_DIM, :n1 - n0] = x[base + n0:base + n1].T
        xsh[:, IN_DIM, :] = 1.0
        xsht = xsh.reshape(128, NPG)

        in_maps.append({
            "xtab": xtab, "srcidx": srcidx, "cidx": cidx,
            "closen2": closen2, "prevn2": prevn2, "xsht": xsht,
            "w4": w4,
        })
    _CACHE.setdefault(
        "Ls", [int(-(-(v + 2) // 16) * 16) for v in ls_max])
    return in_maps


# ---------------------------------------------------------------- runner
class SpmdRunner:
    def __init__(self, nc, n_cores):
        import jax
        from jax.sharding import Mesh, PartitionSpec
        from jax.experimental.shard_map import shard_map
        from concourse.bass2jax import (
            _bass_exec_p, install_neuronx_cc_hook, partition_id_tensor)

        install_neuronx_cc_hook()
        self.n_cores = n_cores
        pname = nc.partition_id_tensor.name if nc.partition_id_tensor else None
        in_names, out_names, out_avals, zero_outs = [], [], [], []
        for alloc in nc.m.functions[0].allocations:
            if not isinstance(alloc, mybir.MemoryLocationSet):
                continue
            name = alloc.memorylocations[0].name
            if alloc.kind == "ExternalInput":
                if name != pname:
                    in_names.append(name)
            elif alloc.kind == "ExternalOutput":
                out_names.append(name)
                shape = tuple(alloc.tensor_shape)
                dt_np = mybir.dt.np(alloc.dtype)
                out_avals.append(jax.core.ShapedArray(shape, dt_np))
                zero_outs.append(np.zeros(shape, dt_np))
        self.in_names, self.out_names = in_names, out_names
        self.zero_outs = zero_outs
        n_params, n_outs = len(in_names), len(out_names)
        all_names = in_names + out_names + ([pname] if pname else [])

        def _body(*args):
            operands = list(args)
            if pname is not None:
                operands.append(partition_id_tensor())
            return tuple(_bass_exec_p.bind(
                *operands, out_avals=tuple(out_avals),
                in_names=tuple(all_names), out_names=tuple(out_names),
                lowering_input_output_aliases=(),
                sim_require_finite=True, sim_require_nnan=True, nc=nc))

        devices = jax.devices()[:n_cores]
        mesh = Mesh(np.asarray(devices), ("core",))
        self._mesh = mesh
        specs_in = (PartitionSpec("core"),) * (n_params + n_outs)
        specs_out = (PartitionSpec("core"),) * n_outs
        self._fn = jax.jit(
            shard_map(_body, mesh=mesh, in_specs=specs_in,
                      out_specs=specs_out, check_rep=False),
            keep_unused=True)
        self._jax = jax

    def prepare(self, in_maps):
        per = [[np.asarray(m[n]) for n in self.in_names] for m in in_maps]
        cat = [np.concatenate([per[c][i] for c in range(self.n_cores)], axis=0)
               for i in range(len(self.in_names))]
        cat += [np.concatenate([z] * self.n_cores, axis=0)
                for z in self.zero_outs]
        return cat

    def device_put(self, args):
        """Ship prepared args to the devices once (for repeat timing)."""
        import jax
        from jax.sharding import NamedSharding, PartitionSpec
        mesh = self._mesh
        sh = NamedSharding(mesh, PartitionSpec("core"))
        out = [jax.device_put(a, sh) for a in args]
        jax.block_until_ready(out)
        return out

    def run(self, args):
        outs = self._fn(*args)
        self._jax.block_until_ready(outs)
        return outs

    def results(self, outs):
        res = [dict() for _ in range(self.n_cores)]
        for i, name in enumerate(self.out_names):
            for c, part in enumerate(
                    np.split(np.asarray(outs[i]), self.n_cores, axis=0)):
                res[c][name] = part
        return res


_CACHE = {}


def kernel(x, edge_index, W_l, b_l, W_r):
    in_maps = prepare_inputs(x, edge_index, W_l, b_l, W_r)
    if "runner" not in _CACHE:
        nc = build_program()
        _CACHE["runner"] = SpmdRunner(nc, N_CORES)
    runner = _CACHE["runner"]
    args = runner.prepare(in_maps)
    res = runner.results(runner.run(args))
    out = np.empty((N_NODES, HIDDEN), np.float32)
    d_l = np.arange(NODES_PC)
    n_map = (d_l // GN) * NPG + d_l % GN          # local node -> padded slot
    for c in range(N_CORES):
        od = res[c]["out"]                        # [128, NJ*16]
        # padded node n at partition n%128, col-block n//128
        full = od.reshape(128, NJ, HIDDEN).transpose(1, 0, 2).reshape(
            NPAD, HIDDEN)
        out[c * NODES_PC:(c + 1) * NODES_PC] = full[n_map]
    return out


# revision 32
# speedup vs baseline: 1.0760x; 1.0205x over previous
"""SAGEConv-style GNN message passing on 8 Trainium2 NeuronCores.

out = (mean_{j in N(i)} x_j) @ W_l + b_l + x_i @ W_r
with N(i) defined by edge_index ([2, E]: src=row0, dst=row1), mean over
in-edges (segment mean by dst), N=100000 nodes, E=6400000 edges.

Distribution: shard by DESTINATION node range — core c owns nodes
[c*12500, (c+1)*12500) and receives exactly the edges targeting its range.
No collective needed.

Device algorithm (v2 — on-chip gather, no per-edge DMA):
  The src space is split into NS=20 sub-tables of 5000 nodes.  Each core's
  edges are bucketed host-side by (sub-table s, dst-group g) where the
  core's 12500 dst nodes are split into 8 groups of 1664; within a bucket
  edges stay dst-sorted.  Per stream s:
    1. DMA the feature-major sub-table [128, 5008] (partition 16g+r holds
       feature r of the sub-table, replicated per group) and int16 edge
       src indices.
    2. gpsimd.ap_gather pulls every edge's src features in ONE instruction
       per stream (each 16-partition group gathers its own edge slice).
    3. Unsegmented fp32 cumsum along the edge axis (tensor_tensor_scan,
       data0 = ones broadcast via a step-0 AP).
    4. gpsimd.ap_gather of the cumsum at per-node segment-close positions;
       consecutive differences give per-node per-stream partial sums.
       Gathered tiles are accumulated across streams (sum of diffs =
       diff of sums) into GACC.
  Epilogue: SUM = diff(GACC); degree from shipped close/prev close index
  tensors (device subtract); out = (SUM @ W_l) * rcp + x @ W_r + b_l via
  PE matmuls (features already partition-major) + one STT per 128-node
  tile.  Host work is layout only: bucketing/sorting edges, packing int16
  index tensors, replicating weights.
"""

import numpy as np

import concourse.bass as bass
import concourse.tile as tile
from concourse import bacc, mybir

# ---------------------------------------------------------------- config
N_NODES = 100000
N_EDGES = 6400000
IN_DIM = 10
HIDDEN = 16
N_CORES = 8

NODES_PC = N_NODES // N_CORES   # 12500 dst nodes per core
GROUPS = 8                      # dst groups per core (16 partitions each)
NPG = 1664                      # dst node SLOTS per group (13*128)
GN = 1568                       # real dst nodes per group (balanced load)
NPAD = GROUPS * NPG             # 13312 padded dst nodes per core
TILES = NPG // 128              # 13 epilogue tiles per group
NJ = NPAD // 128                # 104 output cols per partition (N2 layout)

NS = 16                         # src sub-tables (streams)
SUB = N_NODES // NS             # 6250 src nodes per sub-table
TABW = SUB + 8                  # table width: 6250 rows + zero row + pad
ZROW = SUB                      # zero-row index (gathers 0.0)
LMAX = 7168                     # max edge slots per (stream, group)
SIW = LMAX // 16                # src idx tile free width (DRAM layout)
NCL = 1680                      # close entries per group: 105*16
CIW = NCL // 16


# ---------------------------------------------------------------- device
def build_program(num_devices=N_CORES, Ls=None):
    if Ls is None:
        Ls = _CACHE.get("Ls", [LMAX] * NS)
    P = 128
    nc = bacc.Bacc("TRN2", target_bir_lowering=False, debug=False,
                   num_devices=num_devices)

    xtab = nc.dram_tensor("xtab", [NS, P, TABW], mybir.dt.float32,
                          kind="ExternalInput")
    srcidx = nc.dram_tensor("srcidx", [NS, P, SIW], mybir.dt.int16,
                            kind="ExternalInput")
    cidx = nc.dram_tensor("cidx", [NS, P, CIW], mybir.dt.int16,
                          kind="ExternalInput")
    closen2 = nc.dram_tensor("closen2", [NS, P, NJ], mybir.dt.float32,
                             kind="ExternalInput")
    prevn2 = nc.dram_tensor("prevn2", [NS, P, NJ], mybir.dt.float32,
                            kind="ExternalInput")
    xsht = nc.dram_tensor("xsht", [P, NPG], mybir.dt.float32,
                          kind="ExternalInput")
    # weights stacked per 32-partition quadrant: cols 0:16 = W_l for the even
    # group of the pair, 16:32 = W_l for the odd group, 32:48/48:64 likewise
    # for [W_r; b_l] (row 10 = b_l meets the ones row of xsht).
    w4 = nc.dram_tensor("w4", [P, 4 * HIDDEN], mybir.dt.float32,
                        kind="ExternalInput")
    out_d = nc.dram_tensor("out", [P, NJ * HIDDEN], mybir.dt.float32,
                           kind="ExternalOutput")

    with tile.TileContext(nc) as tc:
        with (
            tc.tile_pool(name="tab", bufs=2) as tab_pool,
            tc.tile_pool(name="msg", bufs=2) as msg_pool,
            tc.tile_pool(name="scan", bufs=1) as scan_pool,
            tc.tile_pool(name="idx", bufs=2) as idx_pool,
            tc.tile_pool(name="g", bufs=2) as g_pool,
            tc.tile_pool(name="epi", bufs=1) as epi_pool,
            tc.tile_pool(name="psum", bufs=8,
                         space=bass.MemorySpace.PSUM) as psum_pool,
        ):
            # prefetch streams 0+1 tables + indices ahead of the statics so
            # the first gathers start as early as possible
            pre = []
            for s in range(2):
                tabp_t = tab_pool.tile([P, TABW], mybir.dt.float32, tag="tab")
                nc.sync.dma_start(tabp_t[:], xtab.ap()[s])
                sip_t = idx_pool.tile([P, SIW], mybir.dt.int16, tag="si")
                nc.sync.dma_start(sip_t[:, :Ls[s] // 16],
                                  srcidx.ap()[s][:, :Ls[s] // 16])
                pre.append((tabp_t, sip_t))

            # ---- static tiles
            ones_t = epi_pool.tile([P, 1], mybir.dt.float32)
            nc.vector.memset(ones_t[:], 1.0)
            gacc_t = epi_pool.tile([P, NCL], mybir.dt.float32)
            nc.vector.memset(gacc_t[:], 0.0)
            dacc_t = epi_pool.tile([P, NJ], mybir.dt.float32)
            nc.vector.memset(dacc_t[:], 0.0)
            pacc_t = epi_pool.tile([P, NJ], mybir.dt.float32)
            nc.vector.memset(pacc_t[:], 0.0)
            xsht_t = epi_pool.tile([P, NPG], mybir.dt.float32)
            nc.sync.dma_start(xsht_t[:], xsht.ap())
            w4_t = epi_pool.tile([P, 4 * HIDDEN], mybir.dt.float32)
            nc.sync.dma_start(w4_t[:], w4.ap())

            # x @ W_r (+ b_l via the ones row) matmuls run during the streams
            # (PE is otherwise idle); results stay resident in PSUM: one bank
            # per group, cols [0,208) reserved for the tail's W_l terms.
            gw = TILES * HIDDEN
            pg_tiles = []
            for g in range(GROUPS):
                q, lo = divmod(g, 2)
                wr_cols = slice(HIDDEN * (2 + lo), HIDDEN * (3 + lo))
                rows = slice(32 * q, 32 * (q + 1))
                pg = psum_pool.tile([P, 2 * gw], mybir.dt.float32, tag="pg")
                pg_tiles.append(pg)
                for t in range(TILES):
                    nc.tensor.matmul(
                        pg[:, gw + HIDDEN * t:gw + HIDDEN * (t + 1)],
                        xsht_t[rows, 128 * t:128 * (t + 1)],
                        w4_t[rows, wr_cols],
                        tile_position=(32 * q, 0))

            # ---- streams
            for s in range(NS):
                ls = Ls[s]
                if s < 2:
                    tab_t, si_t = pre[s]
                else:
                    tab_t = tab_pool.tile([P, TABW], mybir.dt.float32,
                                          tag="tab")
                    nc.sync.dma_start(tab_t[:], xtab.ap()[s])
                    si_t = idx_pool.tile([P, SIW], mybir.dt.int16, tag="si")
                    nc.sync.dma_start(si_t[:, :ls // 16],
                                      srcidx.ap()[s][:, :ls // 16])
                ci_t = idx_pool.tile([P, CIW], mybir.dt.int16, tag="ci")
                nc.sync.dma_start(ci_t[:], cidx.ap()[s])
                cn_t = idx_pool.tile([P, NJ], mybir.dt.float32, tag="cn")
                nc.sync.dma_start(cn_t[:], closen2.ap()[s])
                pn_t = idx_pool.tile([P, NJ], mybir.dt.float32, tag="pn")
                nc.sync.dma_start(pn_t[:], prevn2.ap()[s])

                msg_t = msg_pool.tile([P, LMAX], mybir.dt.float32, tag="msg")
                nc.gpsimd.ap_gather(
                    out_ap=msg_t[:, :ls].rearrange("p (n o) -> p n o", o=1),
                    in_ap=tab_t[:].rearrange("p (n o) -> p n o", o=1),
                    idxs_ap=si_t[:, :ls // 16],
                    channels=P, num_elems=TABW, d=1, num_idxs=ls)

                scan_t = scan_pool.tile([P, LMAX], mybir.dt.float32,
                                        tag="scan")
                d0 = bass.AP(ones_t[:].tensor, ones_t[:].offset,
                             [ones_t[:].ap[0], [0, ls]])
                nc.vector.tensor_tensor_scan(
                    out=scan_t[:, :ls], data0=d0, data1=msg_t[:, :ls],
                    initial=0.0,
                    op0=mybir.AluOpType.mult, op1=mybir.AluOpType.add)

                g_t = g_pool.tile([P, NCL], mybir.dt.float32, tag="g")
                nc.gpsimd.ap_gather(
                    out_ap=g_t[:].rearrange("p (n o) -> p n o", o=1),
                    in_ap=scan_t[:, :ls].rearrange("p (n o) -> p n o", o=1),
                    idxs_ap=ci_t[:],
                    channels=P, num_elems=ls, d=1, num_idxs=NCL)
                nc.vector.tensor_add(out=gacc_t[:], in0=gacc_t[:], in1=g_t[:])
                nc.vector.tensor_add(out=dacc_t[:], in0=dacc_t[:], in1=cn_t[:])
                nc.vector.tensor_add(out=pacc_t[:], in0=pacc_t[:], in1=pn_t[:])

            # ---- epilogue
            sum_t = epi_pool.tile([P, NPG], mybir.dt.float32)
            nc.vector.tensor_tensor(
                out=sum_t[:], in0=gacc_t[:, 1:NPG + 1], in1=gacc_t[:, 0:NPG],
                op=mybir.AluOpType.subtract)
            deg_t = epi_pool.tile([P, NJ], mybir.dt.float32)
            nc.vector.tensor_tensor(
                out=deg_t[:], in0=dacc_t[:], in1=pacc_t[:],
                op=mybir.AluOpType.subtract)
            nc.vector.tensor_scalar_max(deg_t[:], deg_t[:], 1.0)
            rcp_t = epi_pool.tile([P, NJ], mybir.dt.float32)
            nc.vector.reciprocal(rcp_t[:], deg_t[:])

            out_t = epi_pool.tile([P, NJ * HIDDEN], mybir.dt.float32)
            t1_t = epi_pool.tile([P, TILES * HIDDEN], mybir.dt.float32)
            rcpx_t = epi_pool.tile([P, TILES * HIDDEN], mybir.dt.float32)
            for g in range(GROUPS):
                q, lo = divmod(g, 2)
                wl_cols = slice(HIDDEN * lo, HIDDEN * (lo + 1))
                rows = slice(32 * q, 32 * (q + 1))
                pg = pg_tiles[g]
                for t in range(TILES):
                    nc.tensor.matmul(
                        pg[:, HIDDEN * t:HIDDEN * (t + 1)],
                        sum_t[rows, 128 * t:128 * (t + 1)],
                        w4_t[rows, wl_cols],
                        tile_position=(32 * q, 0))
                # rcpx[:, 16t+h] = rcp[:, 13g+t]  (free-dim step-0 broadcast)
                rg = rcp_t[:, TILES * g:TILES * (g + 1)]
                rbc = bass.AP(rg.tensor, rg.offset,
                              [rg.ap[0], rg.ap[1], [0, HIDDEN]])
                nc.scalar.activation(
                    out=rcpx_t[:].rearrange("p (t h) -> p t h", h=HIDDEN),
                    in_=rbc, func=mybir.ActivationFunctionType.Copy)
                # out = p1 * rcpx + p2   (each op reads one PSUM operand)
                nc.vector.tensor_tensor(
                    out=t1_t[:], in0=pg[:, :gw], in1=rcpx_t[:],
                    op=mybir.AluOpType.mult)
                nc.vector.tensor_tensor(
                    out=out_t[:, gw * g:gw * (g + 1)],
                    in0=pg[:, gw:2 * gw], in1=t1_t[:],
                    op=mybir.AluOpType.add)
            nc.sync.dma_start(out_d.ap(), out_t[:])

    nc.compile()
    return nc


# ---------------------------------------------------------------- host
def prepare_inputs(x, edge_index, W_l, b_l, W_r):
    x = np.asarray(x, np.float32)
    W_l = np.asarray(W_l, np.float32)
    b_l = np.asarray(b_l, np.float32)
    W_r = np.asarray(W_r, np.float32)
    src = np.asarray(edge_index[0], np.int64)
    dst = np.asarray(edge_index[1], np.int64)

    # replicated feature-major sub-tables: [NS, 128, TABW]
    # partition 16g+r holds feature r of sub-table s (r<10), zeros otherwise;
    # col ZROW is the zero row.
    xtab = np.zeros((NS, 16, TABW), np.float32)
    for s in range(NS):
        xtab[s, :IN_DIM, :SUB] = x[s * SUB:(s + 1) * SUB].T
    xtab = np.ascontiguousarray(
        np.broadcast_to(xtab[:, None, :, :], (NS, GROUPS, 16, TABW))
    ).reshape(NS, 128, TABW)

    # weights: rows r<10 = W[r]; xsht row 10 is ones so the W_r block gets
    # b_l at row 10.  Stacked [32, 64] quadrant pattern tiled 4x -> [128, 64].
    wlb = np.zeros((16, HIDDEN), np.float32)
    wlb[:IN_DIM] = W_l
    wrb = np.zeros((16, HIDDEN), np.float32)
    wrb[:IN_DIM] = W_r
    wrb[IN_DIM] = b_l
    w32 = np.zeros((32, 4 * HIDDEN), np.float32)
    w32[:16, 0:16] = wlb
    w32[16:, 16:32] = wlb
    w32[:16, 32:48] = wrb
    w32[16:, 48:64] = wrb
    w4 = np.tile(w32, (4, 1))

    order = np.argsort(dst, kind="stable")
    dst_s = dst[order]
    src_s = src[order]
    core_bounds = np.searchsorted(dst_s, np.arange(0, N_NODES + 1, NODES_PC))
    ls_max = np.zeros(NS, np.int64)      # per-stream tight L across cores

    def wrap16(a):
        # [..., K*16] entry-order -> [..., 16, K] partition-wrapped
        shp = a.shape[:-1]
        k = a.shape[-1] // 16
        return np.swapaxes(a.reshape(*shp, k, 16), -1, -2)

    in_maps = []
    for c in range(N_CORES):
        lo, hi = core_bounds[c], core_bounds[c + 1]
        d_l = dst_s[lo:hi] - c * NODES_PC          # [Ec] in [0, 12500)
        sc = src_s[lo:hi]
        s_id = sc // SUB                            # stream
        g_id = d_l // GN                            # dst group (balanced)
        src_loc = (sc - s_id * SUB).astype(np.int16)
        key = (s_id * GROUPS + g_id).astype(np.int64)

        korder = np.argsort(key, kind="stable")     # dst order kept in bucket
        key_s = key[korder]
        srcl_s = src_loc[korder]
        dl_s = d_l[korder]

        cnts = np.bincount(key_s, minlength=NS * GROUPS)
        assert cnts.max() <= LMAX - 8, f"bucket overflow: {cnts.max()}"
        np.maximum(ls_max, cnts.reshape(NS, GROUPS).max(axis=1), out=ls_max)
        starts = np.zeros(NS * GROUPS, np.int64)
        np.cumsum(cnts[:-1], out=starts[1:])
        rank = np.arange(key_s.shape[0]) - starts[key_s]

        # src idx slots: col 0 zero slot, edges at cols 1..cnt, pads ZROW
        A = np.full((NS * GROUPS, LMAX), ZROW, np.int16)
        A[key_s, rank + 1] = srcl_s
        srcidx = wrap16(A.reshape(NS, GROUPS, LMAX)).reshape(NS, 128, SIW)

        # per-node close cols: cumcount within bucket (0 => zero slot)
        node_key = key_s * NPG + (dl_s - g_id[korder] * GN)
        ncnt = np.bincount(node_key, minlength=NS * GROUPS * NPG)
        closes = np.cumsum(
            ncnt.reshape(NS * GROUPS, NPG), axis=1)     # [160, 1664]
        entries = np.full((NS * GROUPS, NCL), 0, np.int16)
        entries[:, 1:NPG + 1] = closes
        entries[:, NPG + 1:] = closes[:, -1:]
        cidx = wrap16(entries.reshape(NS, GROUPS, NCL)).reshape(NS, 128, CIW)

        # N2 close/prev tensors (f32): node n at partition n%128, col n//128
        cl3 = closes.reshape(NS, GROUPS, NPG).astype(np.float32)
        pr3 = np.concatenate(
            [np.zeros((NS, GROUPS, 1), np.float32), cl3[:, :, :-1]], axis=2)
        # [NS, GROUPS*NPG] -> [NS, NJ, 128] -> [NS, 128, NJ]
        closen2 = np.swapaxes(cl3.reshape(NS, NPAD // 128, 128), 1, 2).copy()
        prevn2 = np.swapaxes(pr3.reshape(NS, NPAD // 128, 128), 1, 2).copy()

        # xsht: partition 16g+r = feature r (row 10 = ones) of nodes
        # g*GN..; pad slots zero (row 10 stays 1 -> out = b_l there, unused)
        xsh = np.zeros((GROUPS, 16, NPG), np.float32)
        base = c * NODES_PC
        for g in range(GROUPS):
            n0 = g * GN
            n1 = min((g + 1) * GN, NODES_PC)
            if n1 > n0:
                xsh[g, :IN_DIM, :n1 - n0] = x[base + n0:base + n1].T
        xsh[:, IN_DIM, :] = 1.0
        xsht = xsh.reshape(128, NPG)

        in_maps.append({
            "xtab": xtab, "srcidx": srcidx, "cidx": cidx,
            "closen2": closen2, "prevn2": prevn2, "xsht": xsht,
            "w4": w4,
        })
    _CACHE.setdefault(
        "Ls", [int(-(-(v + 2) // 16) * 16) for v in ls_max])
    return in_maps


# ---------------------------------------------------------------- runner
class SpmdRunner:
    def __init__(self, nc, n_cores):
        import jax
        from jax.sharding import Mesh, PartitionSpec
        from jax.experimental.shard_map import shard_map
        from concourse.bass2jax import (
            _bass_exec_p, install_neuronx_cc_hook, partition_id_tensor)

        install_neuronx_cc_hook()
        self.n_cores = n_cores
        pname = nc.partition_id_tensor.name if nc.partition_id_tensor else None
        in_names, out_names, out_avals, zero_outs = [], [], [], []
        for alloc in nc.m.functions[0].allocations:
            if not isinstance(alloc, mybir.MemoryLocationSet):
                continue
            name = alloc.memorylocations[0].name
            if alloc.kind == "ExternalInput":
                if name != pname:
                    in_names.append(name)
            elif alloc.kind == "ExternalOutput":
                out_names.append(name)
                shape = tuple(alloc.tensor_shape)
                dt_np = mybir.dt.np(alloc.dtype)
                out_avals.append(jax.core.ShapedArray(shape, dt_np))
                zero_outs.append(np.zeros(shape, dt_np))
        self.in_names, self.out_names = in_names, out_names
        self.zero_outs = zero_outs
        n_params, n_outs = len(in_names), len(out_names)
        all_names = in_names + out_names + ([pname] if pname else [])

        def _body(*args):
            operands = list(args)
            if pname is not None:
                operands.append(partition_id_tensor())
            return tuple(_bass_exec_p.bind(
                *operands, out_avals=tuple(out_avals),
                in_names=tuple(all_names), out_names=tuple(out_names),
                lowering_input_output_aliases=(),
                sim_require_finite=True, sim_require_nnan=True, nc=nc))

        devices = jax.devices()[:n_cores]
        mesh = Mesh(np.asarray(devices), ("core",))
        self._mesh = mesh
        specs_in = (PartitionSpec("core"),) * (n_params + n_outs)
        specs_out = (PartitionSpec("core"),) * n_outs
        self._fn = jax.jit(
            shard_map(_body, mesh=mesh, in_specs=specs_in,
                      out_specs=specs_out, check_rep=False),
            keep_unused=True)
        self._jax = jax

    def prepare(self, in_maps):
        per = [[np.asarray(m[n]) for n in self.in_names] for m in in_maps]
        cat = [np.concatenate([per[c][i] for c in range(self.n_cores)], axis=0)
               for i in range(len(self.in_names))]
        cat += [np.concatenate([z] * self.n_cores, axis=0)
                for z in self.zero_outs]
        return cat

    def device_put(self, args):
        """Ship prepared args to the devices once (for repeat timing)."""
        import jax
        from jax.sharding import NamedSharding, PartitionSpec
        mesh = self._mesh
        sh = NamedSharding(mesh, PartitionSpec("core"))
        out = [jax.device_put(a, sh) for a in args]
        jax.block_until_ready(out)
        return out

    def run(self, args):
        outs = self._fn(*args)
        self._jax.block_until_ready(outs)
        return outs

    def results(self, outs):
        res = [dict() for _ in range(self.n_cores)]
        for i, name in enumerate(self.out_names):
            for c, part in enumerate(
                    np.split(np.asarray(outs[i]), self.n_cores, axis=0)):
                res[c][name] = part
        return res


_CACHE = {}


def kernel(x, edge_index, W_l, b_l, W_r):
    in_maps = prepare_inputs(x, edge_index, W_l, b_l, W_r)
    if "runner" not in _CACHE:
        nc = build_program()
        _CACHE["runner"] = SpmdRunner(nc, N_CORES)
    runner = _CACHE["runner"]
    args = runner.prepare(in_maps)
    res = runner.results(runner.run(args))
    out = np.empty((N_NODES, HIDDEN), np.float32)
    d_l = np.arange(NODES_PC)
    n_map = (d_l // GN) * NPG + d_l % GN          # local node -> padded slot
    for c in range(N_CORES):
        od = res[c]["out"]                        # [128, NJ*16]
        # padded node n at partition n%128, col-block n//128
        full = od.reshape(128, NJ, HIDDEN).transpose(1, 0, 2).reshape(
            NPAD, HIDDEN)
        out[c * NODES_PC:(c + 1) * NODES_PC] = full[n_map]
    return out
